# revision 1
# baseline (speedup 1.0000x reference)
"""Trainium2 Bass kernel for AllPassMORRCirculantLinear.

Math (reference, per batch row b):
  xb = x.reshape(bs, q, k); xb = xb*xb
  phi[b,p,q,t] = sum_s xb[b,q,s] * |w|[p,q,(t-s) mod k]   (circular conv, k=8)
  t(phi) = (a^2 + r^2 - 2 a r cos phi) / (1 + (ar)^2 - 2 a r cos phi)
  out[b, p*k+t] = sum_q scale[q] * t(phi[b,p,q,t])

Using t(phi) = 1 - K/(B - 2*rho*cos(phi)) with rho = a*r, B = 1+rho^2,
K = (1-a^2)(1-r^2), and sum_q scale[q] == 0 (scale = [half, -half]):
  out = sum_q s'_q * u_q,   s'_q = -K*scale[q],  u_q = 1/(B - 2 rho cos phi_q)

Distribution: data-parallel over batch across 8 cores (128 rows each).

Per core, per q (pairs of q flow through [128, 2048] tiles):
  PE    : psi = phi + pi/2 in PSUM via TWO accumulating fp16 matmuls
          (hi/lo split, 22-bit effective mantissa; fp32 matmuls stream at
          4 cyc/col and were the bottleneck). The pi/2 shift rides a
          constant stationary/moving row pair a*b ~= pi/2.
  DVE   : psi -> r in [-pi, pi]  (custom DVE op REDUCE_2PI_ANT: magic-number
          round + 2-term Cody-Waite; ACT's Sin spline is only valid for
          |arg| < ~3.5)
  ACT   : sin(r) = cos(phi); then Identity-affine d' = (B - 2 rho c)/s'_q
          (per-q scale/bias APs; Sin+Identity+Square live in one table set)
  DVE   : u' = 1/d' = s'_q * u  (custom fast reciprocal, ~51 ULP)
  Pool  : acc += u' (plain tensor_add; ~3/16 of adds go to DVE to balance)
Sin and the fast reciprocal run at quad width ([128, 4096] tiles, 4 q each)
to amortize per-instruction overhead. All four engines land at ~280 us of
work per core; measured ~300-360 us per pass on hardware (loop-contrast).
"""

import sys

for _p in ("/opt/trn_rl_repo",):
    if _p not in sys.path:
        sys.path.insert(0, _p)

import numpy as np
from contextlib import ExitStack

MRR_A = 0.8682
MRR_R = 0.8602
RHO = MRR_A * MRR_R
BCONST = 1.0 + RHO * RHO
KCONST = (1.0 - MRR_A * MRR_A) * (1.0 - MRR_R * MRR_R)
PI = float(np.pi)

BS, IN_CH, OUT_CH, KB = 1024, 1024, 1024, 8
Q = IN_CH // KB    # 128
P = OUT_CH // KB   # 128
NCORES = 8
BSC = BS // NCORES  # 128 batch rows per core

# range-reduction constants (2-term Cody-Waite, k exact up to 2^9)
MAGIC = 12582912.0  # 1.5 * 2**23: y + MAGIC - MAGIC == round(y) in fp32 RNE
INV2PI = float(np.float32(1.0 / (2.0 * np.pi)))
_tp = 2.0 * np.pi
C_HI = float((np.float32(_tp).view(np.uint32) & np.uint32(0xFFFFFF00)).view(np.float32))
C_LO = float(np.float32(_tp - np.float64(C_HI)))

# fp16 pair with a*b ~= pi/2 (error ~3e-7); a goes in the stationary ones
# row, b in the moving pi/2 row, their exact product lands in fp32 PSUM
ONES_A = 0.9580078125
ONES_B = 1.6396484375

_CACHE = {}


def _reduce_ref(in0, in1, s0, s1, imm2):
    f = np.float32
    y = (in0.astype(f) * f(s0)).astype(f)
    t = (y + in1.astype(f)).astype(f)
    k = (t - in1.astype(f)).astype(f)
    r = (in0.astype(f) - (k * f(s1)).astype(f)).astype(f)
    return (r - (k * f(imm2)).astype(f)).astype(f)


def _register_reduce2pi():
    """Custom DVE op: r = x - 2*pi*round(x/(2*pi)), |r| <= pi (+eps).
    7 ALU stages; in1 must stream a full tile of MAGIC."""
    from concourse import dve_ops
    from concourse.dve_spec import Spec, Src0, Src1, C0, C1, C2, lower
    from concourse.dve_uop import DveOpSpec

    name = "REDUCE_2PI_ANT"
    if name in dve_ops._SUB_OPCODE_FOR_NAME:
        return next(op for op in dve_ops.OPS if op.name == name)
    y = Src0 * C0
    t = y + Src1
    k = t - Src1
    spec = Spec(body=(Src0 - k * C1) - k * C2, reference=_reduce_ref)
    row = max(dve_ops._SUB_OPCODE_FOR_NAME.values()) + 1
    assert row < 0x20
    dve_ops._SUB_OPCODE_FOR_NAME[name] = row
    shas = {}
    for ver in ("v3", "v4"):
        c = DveOpSpec(name=name, opcode=row, uops=lower(spec, ver=ver), rd1_en=True)
        shas[ver] = c.sha(ver)
    op = dve_ops.DveOp(name, spec, subdim=False, uops_sha=shas)
    dve_ops.OPS.append(op)
    dve_ops.CUSTOM_DVE_SPECS[name] = spec
    return op


def _build_nc(niter=1, extra=(0, 0, 0, 0)):
    from concourse import bacc, mybir
    import concourse.tile as tile
    from concourse import masks

    reduce_op = _register_reduce2pi()

    nc = bacc.Bacc("TRN2", debug=False)
    f32 = mybir.dt.float32
    AF = mybir.ActivationFunctionType

    f16 = mybir.dt.float16
    x_d = nc.dram_tensor("x", [BSC, IN_CH], f32, kind="ExternalInput")
    wc1_d = nc.dram_tensor("wc1", [KB + 1, Q, OUT_CH], f16, kind="ExternalInput")
    wc2_d = nc.dram_tensor("wc2", [2 * KB + 1, Q, OUT_CH], f16, kind="ExternalInput")
    scA_d = nc.dram_tensor("scA", [BSC, Q], f32, kind="ExternalInput")
    scB_d = nc.dram_tensor("scB", [BSC, Q], f32, kind="ExternalInput")
    ones_d = nc.dram_tensor("ones", [1, 16, 128], f16, kind="ExternalInput")
    out_d = nc.dram_tensor("out", [BSC, OUT_CH], f32, kind="ExternalOutput")

    with tile.TileContext(nc) as tc:
        with ExitStack() as ctx:
            singles = ctx.enter_context(tc.tile_pool(name="singles", bufs=1))
            # phi pairs [128, 2048] = 4 banks; bufs=2 uses all 8 PSUM banks
            psum = ctx.enter_context(tc.tile_pool(name="psum", bufs=2, space="PSUM"))
            wqp = ctx.enter_context(tc.tile_pool(name="wqp", bufs=2))
            qpool = ctx.enter_context(tc.tile_pool(name="qpool", bufs=4))

            ident = singles.tile([128, 128], f32)
            masks.make_identity(nc, ident[:])

            magic = singles.tile([128, 2048], f32)
            nc.gpsimd.memset(magic[:], MAGIC)

            # accumulators: Pool adds into acc_sb, DVE adds into acc_d
            acc_sb = singles.tile([128, OUT_CH], f32)
            nc.gpsimd.memset(acc_sb[:], 0.0)
            acc_d = singles.tile([128, OUT_CH], f32)
            nc.vector.memset(acc_d[:], 0.0)

            scA = singles.tile([128, Q], f32)
            nc.sync.dma_start(scA[:], scA_d.ap())
            scB = singles.tile([128, Q], f32)
            nc.sync.dma_start(scB[:], scB_d.ap())

            x_sb = singles.tile([128, IN_CH], f32)
            nc.sync.dma_start(x_sb[:], x_d.ap())
            # input intensity modulation: x <- x^2 (in place)
            nc.scalar.activation(x_sb[:], x_sb[:], AF.Square)

            # staged squared-transposed x in fp16 hi/lo:
            # rows 0..7 = xh, row 8 = ONES_A, rows 9..16 = xl
            xsts = []
            for g in range(8):
                xst = singles.tile([17, 16, 128], f16, tag=f"xst{g}")
                nc.scalar.dma_start(xst[8:9, :, :], ones_d.ap())
                xsts.append(xst)

            # per-q PE transposes into one PSUM tile per group; ACT copy
            # rounds to fp16 (xh); DVE computes the fp16 residual (xl),
            # which a small DMA moves to partitions 9..16
            xlp = ctx.enter_context(tc.tile_pool(name="xlp", bufs=2))
            for g in range(8):
                xtp = psum.tile([8, 16 * 128], f32, tag="ps")
                for j in range(16):
                    nc.tensor.transpose(
                        xtp[:, j * 128:(j + 1) * 128],
                        x_sb[:, (g * 16 + j) * 8:(g * 16 + j) * 8 + 8],
                        ident[:])
                nc.scalar.copy(xsts[g][0:8, :, :], xtp[:])
                xl_tmp = xlp.tile([8, 16 * 128], f16)
                nc.vector.tensor_sub(xl_tmp[:], xtp[:],
                                     xsts[g][0:8, :, :].rearrange("s j b -> s (j b)"))
                nc.scalar.dma_start(
                    xsts[g][9:17, :, :].rearrange("s j b -> s (j b)"), xl_tmp[:])

            def run_iter(first):
                # Pool restarts the accumulator each iteration: the first
                # add per iteration uses tensor_copy semantics via memset.
                if not first:
                    nc.gpsimd.memset(acc_sb[:], 0.0)
                    nc.vector.memset(acc_d[:], 0.0)
                for g in range(8):
                  for c8 in range(2):
                    q8 = g * 16 + c8 * 8
                    wq1 = wqp.tile([9, 8, OUT_CH], f16, tag="wq1")
                    nc.sync.dma_start(wq1[:], wc1_d.ap()[:, q8:q8 + 8, :])
                    wq2 = wqp.tile([17, 8, OUT_CH], f16, tag="wq2")
                    nc.sync.dma_start(wq2[:], wc2_d.ap()[:, q8:q8 + 8, :])
                    for qd in range(2):
                        # one quad tile [128, 4096] flows through sin/recip at
                        # quad width; reduce runs per pair, affine per q
                        quad = qpool.tile([128, 4 * OUT_CH], f32, tag="pr")
                        for p2 in range(2):
                            q0 = q8 + qd * 4 + p2 * 2
                            phi = psum.tile([128, 2 * OUT_CH], f32, tag="ps")
                            for half in range(2):
                                jj = c8 * 8 + qd * 4 + p2 * 2 + half
                                for h in range(2):
                                    dst = phi[:, half * OUT_CH + h * 512:
                                              half * OUT_CH + (h + 1) * 512]
                                    nc.tensor.matmul(
                                        dst, xsts[g][0:9, jj, :],
                                        wq1[:, qd * 4 + p2 * 2 + half,
                                            h * 512:(h + 1) * 512],
                                        start=True, stop=False,
                                        skip_group_check=True,
                                        tile_position=(0, 0),
                                    )
                                    nc.tensor.matmul(
                                        dst, xsts[g][0:17, jj, :],
                                        wq2[:, qd * 4 + p2 * 2 + half,
                                            h * 512:(h + 1) * 512],
                                        start=False, stop=True,
                                        skip_group_check=True,
                                        tile_position=(0, 0),
                                    )
                            nc.vector._custom_dve(
                                reduce_op,
                                out=quad[:, p2 * 2 * OUT_CH:(p2 + 1) * 2 * OUT_CH],
                                in0=phi[:], in1=magic[:],
                                s0=INV2PI, s1=C_HI, imm2=C_LO)
                        nc.scalar.activation(quad[:], quad[:], AF.Sin,
                                             bias=0.0, scale=1.0)
                        for jj4 in range(4):
                            q = q8 + qd * 4 + jj4
                            cph = quad[:, jj4 * OUT_CH:(jj4 + 1) * OUT_CH]
                            nc.scalar.activation(cph, cph, AF.Identity,
                                                 bias=scB[:, q:q + 1],
                                                 scale=scA[:, q:q + 1])
                        nc.vector.reciprocal_approx_fast(out=quad[:], in_=quad[:])
                        pi_ = g * 4 + c8 * 2 + qd
                        for jj4 in range(4):
                            src = quad[:, jj4 * OUT_CH:(jj4 + 1) * OUT_CH]
                            if jj4 == 3 and pi_ % 4 != 0:
                                # ~24 of 128 adds go to DVE to balance Pool
                                nc.vector.tensor_add(acc_d[:], acc_d[:], src)
                            else:
                                nc.gpsimd.tensor_add(acc_sb[:], acc_sb[:], src)

            if niter == 1:
                run_iter(True)
            else:
                with tc.For_i(0, niter, 1):
                    run_iter(False)

            out_sb = singles.tile([128, OUT_CH], f32)
            nc.vector.tensor_add(out_sb[:], acc_sb[:], acc_d[:])
            nc.sync.dma_start(out_d.ap(), out_sb[:])

    nc.compile()
    return nc


def _host_prep(weight, morr_output_scale):
    w = np.abs(np.asarray(weight, dtype=np.float32))   # [P, Q, KB]
    s = morr_output_scale - morr_output_scale.mean()
    half = s[..., :-1, :]                              # [1,1,Q//2,1]
    scale = np.concatenate([half, -half], axis=2)[0, 0, :, 0].astype(np.float32)
    sprime = (-KCONST * scale).astype(np.float32)      # folded -K

    # circulant moving-operand layout: wc[s, q, p*KB+t] = w[p, q, (t-s) % KB]
    wc = np.empty((KB, Q, P * KB), np.float32)
    for sh in range(KB):
        rolled = np.roll(w, sh, axis=2)
        wc[sh] = rolled.transpose(1, 0, 2).reshape(Q, P * KB)

    # fp16 hi/lo split (22-bit effective mantissa through the PE):
    #   phi = xh@wh + xh@wl + xl@wh ; the pi/2 shift rides row 8 as a*b
    wh = wc.astype(np.float16)
    wl = (wc - wh.astype(np.float32)).astype(np.float16)
    wq1 = np.zeros((KB + 1, Q, P * KB), np.float16)
    wq1[:KB] = wh
    wq1[KB] = ONES_B
    wq2 = np.zeros((2 * KB + 1, Q, P * KB), np.float16)
    wq2[:KB] = wl
    wq2[KB + 1:] = wh

    scA = np.broadcast_to((-2.0 * RHO / sprime)[None, :], (BSC, Q)).astype(np.float32)
    scB = np.broadcast_to((BCONST / sprime)[None, :], (BSC, Q)).astype(np.float32)
    return wq1, wq2, np.ascontiguousarray(scA), np.ascontiguousarray(scB)


def kernel(x, weight, morr_output_scale, _trace=False):
    from concourse import bass_utils

    if "nc" not in _CACHE:
        _CACHE["nc"] = _build_nc()
    nc = _CACHE["nc"]

    wq1, wq2, scA, scB = _host_prep(weight, morr_output_scale)
    x = np.ascontiguousarray(np.asarray(x, dtype=np.float32))

    in_maps = []
    for c in range(NCORES):
        in_maps.append({
            "x": np.ascontiguousarray(x[c * BSC:(c + 1) * BSC]),
            "wc1": wq1, "wc2": wq2, "scA": scA, "scB": scB,
            "ones": np.full((1, 16, 128), ONES_A, np.float16),
        })
    res = bass_utils.run_bass_kernel_spmd(
        nc, in_maps, core_ids=list(range(NCORES)), trace=_trace)
    out = np.concatenate([res.results[c]["out"] for c in range(NCORES)], axis=0)
    if _trace:
        _CACHE["last_results"] = res
    return out

